# revision 13
# baseline (speedup 1.0000x reference)
"""Trainium2 Bass kernel for nn_LogicLayer (differentiable logic-gate layer).

Math:
    a = x[:, idx_a]; b = x[:, idx_b]                  # gather columns
    c = softmax(weights) @ T                          # [O, 4] truth-table coeffs
    out = c0*(1-a)(1-b) + c1*(1-a)b + c2*a(1-b) + c3*ab
        = k0 + ka*a + kb*b + kab*a*b
  with k0 = c0, ka = c2-c0, kb = c1-c0, kab = c0-c1-c2+c3.

Device strategy (8 cores, out_dim sharded, 2048 gates/core):
  - Host quantizes x to affine int16 fixed point u = rint((x-m)*s) with
    m = (min+max)/2, s = 65536/(max-min), and pre-transposes to
    uT [in_dim, B] so a gate's input column is a contiguous 8KB row. The
    bilinear form stays bilinear in u with host-folded coefficients:
      out = K0 + KA*ua + KB*ub + KAB*ua*ub
  - Per 128-gate block: gather the 128 a-columns and b-columns of uT into
    SBUF as [128 gates, 4096 batch] int16 tiles (SWDGE dma_gather on two
    queues, or single-queue indirect DMA), then
      t = KAB*ua+KB  /  v = KA*ua+K0   (engine per variant)
      r = t*ub  /  o = r+v -> bf16     (engine per variant)
    and store the [128, 4096] bf16 tile straight to DRAM in [gate, batch]
    layout — no on-device transpose. The host unshard upcasts bf16->f32
    and transposes (exact upcast; bf16 store adds <= 2^-9 relative error,
    total ~4e-3 vs the 2e-2 gate).
  - Per-core DMA: 32MB int16 gather reads + 16MB bf16 writes = 48MB.
"""

import contextlib

import numpy as np

import concourse.bass as bass
import concourse.tile as tile
from concourse import bacc, mybir
from concourse.bass_utils import run_bass_kernel_spmd

B = 4096          # batch
IN_DIM = 4096     # input features
O = 16384         # gates (out_dim)
NCORES = 8
OSH = O // NCORES  # 2048 gates per core
P = 128
GBLOCKS = OSH // P  # 16 gate blocks per core

# Engine assignment per op + DMA routing.
#   t = KAB*a+KB; v = KA*a+K0; r = t*b; o = r+v (bf16 out)
# engines: 'dve' | 'act' | 'gps'; o may be ('dve', 'gps', frac_on_dve)
# gather: 'ind' (indirect DMA, single qPoolDynamic) | 'sw2' (dma_gather on
#         2 SWDGE queues); store: 'sp' | 'split' (alternate SP/Act HWDGE)
VARIANTS = {
    "v1": dict(t="dve", v="act", r="dve", o="dve", gather="ind", store="sp"),
    "v2": dict(t="act", v="act", r="dve", o="dve", gather="ind", store="sp"),
    "v3": dict(t="act", v="act", r="dve", o="dve", gather="sw2", store="split"),
    "v4": dict(t="act", v="act", r="dve", o="gps", gather="sw2", store="split"),
    "v5": dict(
        t="act", v="act", r="dve", o=("dve", "gps", 0.5), gather="sw2", store="split"
    ),
    "v6": dict(
        t="act", v="act", r="dve", o=("dve", "gps", 0.75), gather="sw2", store="split"
    ),
    "v7": dict(
        t="act", v="act", r="dve", o="dve", gather="ind", store="split",
        gath_bufs=6, ot_bufs=3,
    ),
    # one merged a+b gather per block ([128,2] offset AP): halves Pool-queue
    # DMA count, same bytes
    "v8": dict(
        t="act", v="act", r="dve", o="dve", gather="ind2", store="split",
        gath_bufs=6, ot_bufs=3,
    ),
    "v9": dict(
        t="act", v="act", r="dve", o=("dve", "gps", 0.75), gather="ind",
        store="split", gath_bufs=6, ot_bufs=3,
    ),
    "v11": dict(
        t="act", v="act", r="dve", o="dve", gather="ind", store="split",
        gath_bufs=4, tmp_bufs=3, ot_bufs=3,
    ),
    # half-tile stores on both HWDGE rings within every block
    "v12": dict(
        t="act", v="act", r="dve", o="dve", gather="ind", store="split2",
        gath_bufs=6, ot_bufs=3,
    ),
    # hybrid: a via HW-dynamic indirect (Pool q0), b via SWDGE dma_gather
    # (qPoolDynamic1) — splits the 32MB gather stream across two queues
    "v10": dict(
        t="act", v="act", r="dve", o="dve", gather="hyb", store="split",
        gath_bufs=6, ot_bufs=3,
    ),
    # diagnostics: pure-DMA variants (no compute; store ships a_t bits)
    "d_dma": dict(ops=False, gather="ind", store="sp"),
    "d_dma3": dict(ops=False, gather="sw2", store="split"),
}
VARIANT = "v7"

# Sort each core's gates by idx_a so a-gather descriptors walk ascending
# HBM addresses (pure input-data change; NEFF identical). Host unshard
# inverse-permutes the output columns.
SORT_GATES = False
_PERMS = [None] * NCORES

_PROGRAMS = {}


def _build_program(reps=1, variant=None):
    cfg = VARIANTS[variant or VARIANT]
    f32 = mybir.dt.float32
    i32 = mybir.dt.int32
    i16 = mybir.dt.int16
    bf16 = mybir.dt.bfloat16

    swdge = cfg["gather"] == "sw2"
    merged = cfg["gather"] == "ind2"
    hyb = cfg["gather"] == "hyb"
    nc = bacc.Bacc(None, num_swdge_queues=2 if (swdge or hyb) else 1)
    xt_d = nc.declare_dram_parameter("xt", [IN_DIM, B], i16, isOutput=False)
    if merged:
        iab_d = nc.declare_dram_parameter(
            "idxab", [P, GBLOCKS * 2], i32, isOutput=False
        )
    elif hyb:
        ia_d = nc.declare_dram_parameter("idxa", [P, GBLOCKS], i32, isOutput=False)
        ib16_d = nc.declare_dram_parameter(
            "idxb16", [P, GBLOCKS * 8], i16, isOutput=False
        )
    elif swdge:
        ia16_d = nc.declare_dram_parameter(
            "idxa16", [P, GBLOCKS * 8], i16, isOutput=False
        )
        ib16_d = nc.declare_dram_parameter(
            "idxb16", [P, GBLOCKS * 8], i16, isOutput=False
        )
    else:
        ia_d = nc.declare_dram_parameter("idxa", [P, GBLOCKS], i32, isOutput=False)
        ib_d = nc.declare_dram_parameter("idxb", [P, GBLOCKS], i32, isOutput=False)
    coef_d = nc.declare_dram_parameter("coef", [P, GBLOCKS * 4], f32, isOutput=False)
    # output stays in [gate, batch] layout; host transposes during unshard
    out_d = nc.declare_dram_parameter("out", [OSH, B], bf16, isOutput=True)

    def op_engine(name):
        return {"dve": nc.vector, "act": nc.scalar, "gps": nc.gpsimd}[name]

    with tile.TileContext(nc) as tc:
        if swdge or hyb:
            from concourse.library_config import mlp

            nc.gpsimd.load_library(mlp)
        with (
            tc.tile_pool(name="const", bufs=1) as const_pool,
            tc.tile_pool(name="gath", bufs=cfg.get("gath_bufs", 4)) as gath_pool,
            tc.tile_pool(name="tmp", bufs=cfg.get("tmp_bufs", 2)) as tmp_pool,
            tc.tile_pool(name="ot", bufs=cfg.get("ot_bufs", 2)) as ot_pool,
        ):
            if merged:
                idxab_t = const_pool.tile([P, GBLOCKS * 2], i32)
                nc.sync.dma_start(out=idxab_t[:], in_=iab_d[:])
            elif hyb:
                idxa_t = const_pool.tile([P, GBLOCKS], i32)
                nc.sync.dma_start(out=idxa_t[:], in_=ia_d[:])
                idxb_t = const_pool.tile([P, GBLOCKS * 8], i16)
                nc.sync.dma_start(out=idxb_t[:], in_=ib16_d[:])
            elif swdge:
                idxa_t = const_pool.tile([P, GBLOCKS * 8], i16)
                nc.sync.dma_start(out=idxa_t[:], in_=ia16_d[:])
                idxb_t = const_pool.tile([P, GBLOCKS * 8], i16)
                nc.sync.dma_start(out=idxb_t[:], in_=ib16_d[:])
            else:
                idxa_t = const_pool.tile([P, GBLOCKS], i32)
                nc.sync.dma_start(out=idxa_t[:], in_=ia_d[:])
                idxb_t = const_pool.tile([P, GBLOCKS], i32)
                nc.sync.dma_start(out=idxb_t[:], in_=ib_d[:])
            coef_t = const_pool.tile([P, GBLOCKS * 4], f32)
            nc.sync.dma_start(out=coef_t[:], in_=coef_d[:])

            loop_cm = (
                tc.For_i(0, reps, 1) if reps > 1 else contextlib.nullcontext()
            )
            with loop_cm:
                for gb in range(GBLOCKS):
                    if hyb:
                        a_tt = gath_pool.tile([P, B], i16, tag="a")
                        nc.gpsimd.indirect_dma_start(
                            out=a_tt[:],
                            out_offset=None,
                            in_=xt_d[:],
                            in_offset=bass.IndirectOffsetOnAxis(
                                ap=idxa_t[:, gb : gb + 1], axis=0
                            ),
                        )
                        a_t = a_tt[:]
                        b_t3 = gath_pool.tile([P, 1, B], i16, tag="b")
                        nc.gpsimd.dma_gather(
                            b_t3[:],
                            xt_d[:],
                            idxb_t[:, gb * 8 : (gb + 1) * 8],
                            P,
                            P,
                            B,
                            queue_num=1,
                        )
                        b_t = b_t3[:, 0, :]
                    elif merged:
                        g_t = gath_pool.tile([P, 2, B], i16, tag="g")
                        nc.gpsimd.indirect_dma_start(
                            out=g_t[:],
                            out_offset=None,
                            in_=xt_d[:],
                            in_offset=bass.IndirectOffsetOnAxis(
                                ap=idxab_t[:, 2 * gb : 2 * gb + 2], axis=0
                            ),
                        )
                        a_t = g_t[:, 0, :]
                        b_t = g_t[:, 1, :]
                    elif swdge:
                        a_t3 = gath_pool.tile([P, 1, B], i16, tag="a")
                        nc.gpsimd.dma_gather(
                            a_t3[:],
                            xt_d[:],
                            idxa_t[:, gb * 8 : (gb + 1) * 8],
                            P,
                            P,
                            B,
                            queue_num=0,
                        )
                        a_t = a_t3[:, 0, :]
                        b_t3 = gath_pool.tile([P, 1, B], i16, tag="b")
                        nc.gpsimd.dma_gather(
                            b_t3[:],
                            xt_d[:],
                            idxb_t[:, gb * 8 : (gb + 1) * 8],
                            P,
                            P,
                            B,
                            queue_num=1,
                        )
                        b_t = b_t3[:, 0, :]
                    else:
                        a_tt = gath_pool.tile([P, B], i16, tag="a")
                        nc.gpsimd.indirect_dma_start(
                            out=a_tt[:],
                            out_offset=None,
                            in_=xt_d[:],
                            in_offset=bass.IndirectOffsetOnAxis(
                                ap=idxa_t[:, gb : gb + 1], axis=0
                            ),
                        )
                        a_t = a_tt[:]
                        b_tt = gath_pool.tile([P, B], i16, tag="b")
                        nc.gpsimd.indirect_dma_start(
                            out=b_tt[:],
                            out_offset=None,
                            in_=xt_d[:],
                            in_offset=bass.IndirectOffsetOnAxis(
                                ap=idxb_t[:, gb : gb + 1], axis=0
                            ),
                        )
                        b_t = b_tt[:]

                    if not cfg.get("ops", True):
                        # pure-DMA diagnostic: ship the gathered bits out
                        if cfg["store"] == "split":
                            st_eng = nc.sync if gb % 2 == 0 else nc.scalar
                        else:
                            st_eng = nc.sync
                        st_eng.dma_start(
                            out=out_d[gb * P : (gb + 1) * P, :],
                            in_=a_t.bitcast(bf16),
                        )
                        # b_t is gathered but unconsumed; that's fine.
                        continue

                    ap_K0 = coef_t[:, 4 * gb : 4 * gb + 1]
                    ap_KA = coef_t[:, 4 * gb + 1 : 4 * gb + 2]
                    ap_KB = coef_t[:, 4 * gb + 2 : 4 * gb + 3]
                    ap_KAB = coef_t[:, 4 * gb + 3 : 4 * gb + 4]

                    # t = KAB*ua + KB
                    t_t = tmp_pool.tile([P, B], f32, tag="t")
                    if cfg["t"] == "act":
                        nc.scalar.activation(
                            t_t[:],
                            a_t,
                            mybir.ActivationFunctionType.Identity,
                            bias=ap_KB,
                            scale=ap_KAB,
                        )
                    else:
                        op_engine(cfg["t"]).tensor_scalar(
                            t_t[:],
                            a_t,
                            ap_KAB,
                            ap_KB,
                            op0=mybir.AluOpType.mult,
                            op1=mybir.AluOpType.add,
                        )
                    # v = KA*ua + K0
                    v_t = tmp_pool.tile([P, B], f32, tag="v")
                    if cfg["v"] == "act":
                        nc.scalar.activation(
                            v_t[:],
                            a_t,
                            mybir.ActivationFunctionType.Identity,
                            bias=ap_K0,
                            scale=ap_KA,
                        )
                    else:
                        op_engine(cfg["v"]).tensor_scalar(
                            v_t[:],
                            a_t,
                            ap_KA,
                            ap_K0,
                            op0=mybir.AluOpType.mult,
                            op1=mybir.AluOpType.add,
                        )
                    # r = t*ub (in place)
                    op_engine(cfg["r"]).tensor_tensor(
                        out=t_t[:], in0=t_t[:], in1=b_t, op=mybir.AluOpType.mult
                    )
                    # o = r + v -> bf16
                    o_t = ot_pool.tile([P, B], bf16, tag="o")
                    ocfg = cfg["o"]
                    if isinstance(ocfg, tuple):
                        e0, e1, frac = ocfg
                        split = int(B * frac) // 512 * 512
                        op_engine(e0).tensor_tensor(
                            out=o_t[:, :split],
                            in0=t_t[:, :split],
                            in1=v_t[:, :split],
                            op=mybir.AluOpType.add,
                        )
                        op_engine(e1).tensor_tensor(
                            out=o_t[:, split:],
                            in0=t_t[:, split:],
                            in1=v_t[:, split:],
                            op=mybir.AluOpType.add,
                        )
                    else:
                        op_engine(ocfg).tensor_tensor(
                            out=o_t[:], in0=t_t[:], in1=v_t[:], op=mybir.AluOpType.add
                        )
                    if cfg["store"] == "split2":
                        h = B // 2
                        nc.sync.dma_start(
                            out=out_d[gb * P : (gb + 1) * P, :h],
                            in_=o_t[:, :h],
                        )
                        nc.scalar.dma_start(
                            out=out_d[gb * P : (gb + 1) * P, h:],
                            in_=o_t[:, h:],
                        )
                    else:
                        if cfg["store"] == "split":
                            st_eng = nc.sync if gb % 2 == 0 else nc.scalar
                        else:
                            st_eng = nc.sync
                        st_eng.dma_start(
                            out=out_d[gb * P : (gb + 1) * P, :], in_=o_t[:]
                        )
    # Bacc defers register allocation + wait-splitting to compile(); the
    # bass2jax/PJRT path serializes BIR directly, so run it here.
    nc.compile()
    return nc


def _get_program(reps=1, variant=None):
    key = (reps, variant or VARIANT)
    if key not in _PROGRAMS:
        _PROGRAMS[key] = _build_program(reps, variant)
    return _PROGRAMS[key]


def _host_prep(x, weights, idx_a, idx_b):
    x = np.asarray(x, dtype=np.float32)
    xmin = float(x.min())
    xmax = float(x.max())
    m = 0.5 * (xmin + xmax)
    s = 65536.0 / max(xmax - xmin, 1e-12)
    u = np.clip(np.rint((x.astype(np.float64) - m) * s), -32768, 32767)
    xt = np.ascontiguousarray(u.astype(np.int16).T)

    # truth table: T[i, j] = bit (3-j) of i
    tbl = ((np.arange(16)[:, None] >> (3 - np.arange(4))[None, :]) & 1).astype(
        np.float64
    )
    w = np.asarray(weights, dtype=np.float64)
    w = w - w.max(axis=-1, keepdims=True)
    e = np.exp(w)
    p = e / e.sum(axis=-1, keepdims=True)
    c = p @ tbl  # [O, 4]
    k0 = c[:, 0]
    ka = c[:, 2] - c[:, 0]
    kb = c[:, 1] - c[:, 0]
    kab = c[:, 0] - c[:, 1] - c[:, 2] + c[:, 3]

    K0 = k0 + (ka + kb) * m + kab * m * m
    KA = (ka + kab * m) / s
    KB = (kb + kab * m) / s
    KAB = kab / (s * s)
    coef = np.stack([K0, KA, KB, KAB], axis=1).astype(np.float32)  # [O, 4]

    ia = np.asarray(idx_a, dtype=np.int32)
    ib = np.asarray(idx_b, dtype=np.int32)
    return xt, coef, ia, ib


def _swdge_idx(ia_shard):
    """[OSH] int -> [128, GBLOCKS*8] int16 in dma_gather wrap layout:
    within block gb, idx i lives at [i % 16, gb*8 + i // 16] (first 16
    partitions), replicated down the partition dim."""
    w = ia_shard.reshape(GBLOCKS, 8, 16)  # [gb, col, p]
    w16 = np.ascontiguousarray(
        w.transpose(2, 0, 1).reshape(16, GBLOCKS * 8)
    ).astype(np.int16)
    return np.ascontiguousarray(np.tile(w16, (8, 1)))


def make_in_maps(x, weights, idx_a, idx_b):
    xt, coef, ia, ib = _host_prep(x, weights, idx_a, idx_b)
    in_maps = []
    for k in range(NCORES):
        osl = slice(k * OSH, (k + 1) * OSH)
        ia_s, ib_s, coef_s = ia[osl], ib[osl], coef[osl]
        if SORT_GATES:
            perm = np.argsort(ia_s, kind="stable")
            _PERMS[k] = perm
            ia_s, ib_s, coef_s = ia_s[perm], ib_s[perm], coef_s[perm]
        else:
            _PERMS[k] = None
        # swizzle: gate g (within shard) = gb*128 + p  ->  [p, gb]
        ia_k = np.ascontiguousarray(ia_s.reshape(GBLOCKS, P).T)
        ib_k = np.ascontiguousarray(ib_s.reshape(GBLOCKS, P).T)
        # coef: [GBLOCKS, P, 4] -> [P, GBLOCKS, 4] -> [P, GBLOCKS*4]
        coef_k = np.ascontiguousarray(
            coef_s.reshape(GBLOCKS, P, 4).transpose(1, 0, 2).reshape(P, GBLOCKS * 4)
        )
        iab_k = np.ascontiguousarray(
            np.stack([ia_k, ib_k], axis=2).reshape(P, GBLOCKS * 2)
        )
        in_maps.append(
            {
                "xt": xt,
                "idxa": ia_k,
                "idxb": ib_k,
                "idxab": iab_k,
                "idxa16": _swdge_idx(ia_s),
                "idxb16": _swdge_idx(ib_s),
                "coef": coef_k,
            }
        )
    return in_maps


def _unshard(per_core_outs):
    """per_core_outs[k]: bf16 [OSH, B] -> full f32 [B, O]."""
    out = np.empty((B, O), dtype=np.float32)
    for k, r in enumerate(per_core_outs):
        blk = np.asarray(r).astype(np.float32).T  # [B, OSH], sorted gate order
        if _PERMS[k] is not None:
            unp = np.empty_like(blk)
            unp[:, _PERMS[k]] = blk
            blk = unp
        out[:, k * OSH : (k + 1) * OSH] = blk
    return out


def run_kernel(x, weights, idx_a, idx_b, trace=False, variant=None):
    """Returns (out, BassKernelResults)."""
    in_maps = make_in_maps(x, weights, idx_a, idx_b)
    nc = _get_program(1, variant)
    try:
        res = run_bass_kernel_spmd(nc, in_maps, list(range(NCORES)), trace=trace)
    except Exception:
        # transient device/tunnel hiccups: one retry is cheap insurance.
        res = run_bass_kernel_spmd(nc, in_maps, list(range(NCORES)), trace=trace)
    out = _unshard([res.results[k]["out"] for k in range(NCORES)])
    return out, res


def loop_check(out_hi, actual):
    """Sanity diff between the looped program's raw core-0 output and the
    single-shot full result (both deterministic -> should be ~0)."""
    core0 = np.asarray(out_hi[:OSH]).astype(np.float32).T  # [B, OSH]
    if _PERMS[0] is not None:
        unp = np.empty_like(core0)
        unp[:, _PERMS[0]] = core0
        core0 = unp
    return np.abs(core0 - actual[:, :OSH]).max()


def kernel(x, weights, idx_a, idx_b):
    out, _ = run_kernel(x, weights, idx_a, idx_b, trace=False)
    return out
